# revision 32
# baseline (speedup 1.0000x reference)
"""Trainium2 Bass kernel for DeepseekAttention (T=4096, H=2048, 16 heads, d=128).

Tensor-parallel over heads: 8 NeuronCores x 2 heads each (SPMD, per-core inputs).
Host side: transpose hidden -> hidden^T fp16, slice w_qkv/w_o per core, and
precompute RoPE cos/sin tables + causal mask blocks. Per core:
  phase 1: Q^T/K^T = w^T x hidden^T in [d, T] layout (fp16 matmuls), RoPE via
           DVE with rotate-half done by SBUF-to-SBUF DMA partition swaps;
           V produced directly in [t, d] layout (hidden^T tiles stationary).
  phase 2: causal attention in S^T = K Q^T layout, 512-wide q-chunks:
           exp on ACT (no max subtraction needed: scores are O(1)), softmax
           denominator via ones-stationary matmul accumulated in PSUM,
           normalization via DVE reciprocal + GPSIMD partition_broadcast.
           Fully-masked q columns of diagonal k-tiles are skipped.
  phase 3: partial out = A @ w_o rows-slice (software-pipelined one chunk
           behind attention); fp16 partials summed across cores on the host.
"""

import numpy as np

import concourse.tile as tile
from concourse import bacc, mybir
from concourse.bass_utils import run_bass_kernel_spmd

T = 4096
HID = 2048
NHEADS = 16
HD = 128
NCORES = 8
HPC = NHEADS // NCORES        # 2 heads per core
FEAT = HPC * HD               # 256 per-core attention features
QKVF = 3 * FEAT               # 768 per-core qkv features
CH = 512                      # T-chunk width
NCH = T // CH                 # 8 chunks
KT = HID // 128               # 16 hidden k-tiles
FT = QKVF // 128              # 6 qkv feature tiles
SCALE = float(HD) ** -0.5
MASK_NEG = -30000.0

BF = mybir.dt.bfloat16
F16 = mybir.dt.float16
F32 = mybir.dt.float32


def _build_bass():
    nc = bacc.Bacc("TRN2", target_bir_lowering=False, debug=False,
                   num_devices=NCORES)

    hidT = nc.dram_tensor("hidT", [HID, T], F16, kind="ExternalInput").ap()
    wqkv = nc.dram_tensor("wqkv", [HID, QKVF], F16, kind="ExternalInput").ap()
    wo = nc.dram_tensor("wo", [FEAT, HID], F16, kind="ExternalInput").ap()
    cos2 = nc.dram_tensor("cos2", [128, T], F16, kind="ExternalInput").ap()
    sin2 = nc.dram_tensor("sin2", [128, T], F16, kind="ExternalInput").ap()
    masks = nc.dram_tensor("masks", [128, 4 * CH], F32, kind="ExternalInput").ap()
    out = nc.dram_tensor("out", [T, HID], F16, kind="ExternalOutput").ap()

    with tile.TileContext(nc) as tc:
        _emit(tc, hidT, wqkv, wo, cos2, sin2, masks, out)
    nc.compile()
    return nc


def _emit(tc, hidT, wqkv, wo, cos2, sin2, masks, out):
    nc = tc.nc
    from contextlib import ExitStack
    ctx = ExitStack()
    with ctx:
        const = ctx.enter_context(tc.tile_pool(name="const", bufs=1))
        hidp = ctx.enter_context(tc.tile_pool(name="hidp", bufs=2))
        rawp = ctx.enter_context(tc.tile_pool(name="rawp", bufs=6))
        ropep = ctx.enter_context(tc.tile_pool(name="ropep", bufs=4))
        persist = ctx.enter_context(tc.tile_pool(name="persist", bufs=1))
        ptp = ctx.enter_context(tc.tile_pool(name="ptp", bufs=6))
        smallp = ctx.enter_context(tc.tile_pool(name="smallp", bufs=2))
        stgp = ctx.enter_context(tc.tile_pool(name="stgp", bufs=2))
        # PSUM: 8 banks -> S/pb/wo 4, O+D 2, qkv 2 (released after phase 1,
        # its banks become the deeper late-attention O+D pool)
        psmm = ctx.enter_context(tc.tile_pool(name="psmm", bufs=4, space="PSUM"))
        pswo = psmm
        pso_cm = tc.tile_pool(name="pso", bufs=2, space="PSUM")
        pso = pso_cm.__enter__()
        psd = pso
        psqkv_cm = tc.tile_pool(name="psqkv", bufs=2, space="PSUM")
        psqkv = psqkv_cm.__enter__()

        # ---- constants ----
        ones_col = const.tile([128, 1], BF, tag="ones_col")
        nc.any.memset(ones_col[:], 1.0)
        wqkv_sb = const.tile([128, KT * QKVF], F16, tag="wqkv_sb")
        for kt in range(KT):
            nc.scalar.dma_start(wqkv_sb[:, kt * QKVF:(kt + 1) * QKVF],
                                wqkv[kt * 128:(kt + 1) * 128, :])
        cos_sb = const.tile([128, T], F16, tag="cos_sb")
        sin_sb = const.tile([128, T], F16, tag="sin_sb")
        mask_sb = const.tile([128, 4 * CH], F32, tag="mask_sb")
        nc.scalar.dma_start(cos_sb[:], cos2[:])
        nc.scalar.dma_start(sin_sb[:], sin2[:])
        nc.scalar.dma_start(mask_sb[:], masks[:])
        wo_sb = []
        for h in range(HPC):
            t = const.tile([128, HID], F16, tag=f"wo_sb{h}", name=f"wo_sb{h}")
            nc.scalar.dma_start(t[:], wo[h * 128:(h + 1) * 128, :])
            wo_sb.append(t)

        # ---- persistent activation tiles ----
        QTR = [[persist.tile([128, CH], F16, tag=f"qtr{h}_{c}", name=f"qtr{h}_{c}")
                for c in range(NCH)] for h in range(HPC)]
        KTR = [[persist.tile([128, CH], F16, tag=f"ktr{h}_{c}", name=f"ktr{h}_{c}")
                for c in range(NCH)] for h in range(HPC)]
        VV = persist.tile([128, HPC * T], BF, tag="vv", name="vv")
        AT = [[persist.tile([128, CH], F16, tag=f"at{h}_{c}", name=f"at{h}_{c}")
               for c in range(NCH)] for h in range(HPC)]

        # ================= phase 1: QKV^T projection + RoPE + V transpose ====
        for c in range(NCH):
            hid_sb = hidp.tile([128, KT * CH], F16, tag="hid", name=f"hid{c}")
            if c == 0:
                for kt in range(KT):
                    nc.sync.dma_start(
                        hid_sb[:, kt * CH:(kt + 1) * CH],
                        hidT[kt * 128:(kt + 1) * 128, c * CH:(c + 1) * CH])
            else:
                hid_v = hidT[:, c * CH:(c + 1) * CH].rearrange(
                    "(kt p) t -> p kt t", p=128)
                nc.sync.dma_start(
                    hid_sb[:].rearrange("p (kt t) -> p kt t", kt=KT), hid_v)

            for ft in range(4):
                # Q^T (ft 0,1) and K^T (ft 2,3) in [d, T] layout -> RoPE
                ps = psqkv.tile([128, CH], F32, tag="mmA", name=f"psqkv{c}_{ft}")
                for kt in range(KT):
                    nc.tensor.matmul(
                        ps[:],
                        wqkv_sb[:, kt * QKVF + ft * 128: kt * QKVF + (ft + 1) * 128],
                        hid_sb[:, kt * CH:(kt + 1) * CH],
                        start=(kt == 0), stop=(kt == KT - 1))
                h = ft % 2
                raw = rawp.tile([128, CH], F16, tag="raw", name=f"raw{c}_{ft}")
                nc.scalar.copy(raw[:], ps[:])
                rot = ropep.tile([128, CH], F16, tag="rot", name=f"rot{c}_{ft}")
                nc.sync.dma_start(rot[0:64, :], raw[64:128, :])
                nc.sync.dma_start(rot[64:128, :], raw[0:64, :])
                ta = ropep.tile([128, CH], F16, tag="ta", name=f"ta{c}_{ft}")
                tb = ropep.tile([128, CH], F16, tag="tb", name=f"tb{c}_{ft}")
                csl = slice(c * CH, (c + 1) * CH)
                nc.vector.tensor_mul(ta[:], raw[:], cos_sb[:, csl])
                nc.vector.tensor_mul(tb[:], rot[:], sin_sb[:, csl])
                dst = QTR[h][c] if ft < 2 else KTR[h][c]
                nc.vector.tensor_add(dst[:], ta[:], tb[:])

            # V in [t, d] layout directly: lhsT = hidT tile, rhs = w_v cols
            for j in range(4):
                psv = psqkv.tile([128, 2 * 128], F32, tag="mmA",
                                 name=f"psv{c}_{j}")
                for kt in range(KT):
                    nc.tensor.matmul(
                        psv[:],
                        hid_sb[:, kt * CH + j * 128: kt * CH + (j + 1) * 128],
                        wqkv_sb[:, kt * QKVF + 512: kt * QKVF + 768],
                        start=(kt == 0), stop=(kt == KT - 1))
                kt_ = 4 * c + j
                nc.scalar.copy(VV[:, kt_ * 256:(kt_ + 1) * 256], psv[:])


        # ======= phase 2+3: causal attention interleaved with w_o, per chunk ==
        psod2 = None
        for c in range(NCH):
            nkt = 4 * (c + 1)
            if c == 4:
                psqkv_cm.__exit__(None, None, None)
                pso_cm.__exit__(None, None, None)
                psod2 = ctx.enter_context(
                    tc.tile_pool(name="psod2", bufs=4, space="PSUM"))
            for h in range(HPC):
                odp = pso if c < 4 else psod2
                pd = odp.tile([1, CH], F32, tag="o2" if c >= 4 else "o",
                              name=f"pd{h}_{c}")
                po = odp.tile([128, CH], F32, tag="o2" if c >= 4 else "o",
                              name=f"po{h}_{c}")
                for kt in range(nkt):
                    r = kt - 4 * c
                    qo = 128 * r if r > 0 else 0   # skip fully-masked q cols
                    ps = psmm.tile([128, CH], F32, tag="mm", name=f"pss{h}_{c}_{kt}")
                    nc.tensor.matmul(
                        ps[:, qo:],
                        KTR[h][kt // 4][:, (kt % 4) * 128:(kt % 4 + 1) * 128],
                        QTR[h][c][:, qo:],
                        start=True, stop=True)
                    if r >= 0:
                        # mask only affects q in [128r, 128r+128) of this block
                        nc.vector.tensor_add(
                            ps[:, qo:qo + 128], ps[:, qo:qo + 128],
                            mask_sb[:, r * CH + qo:r * CH + qo + 128])
                    pt = ptp.tile([128, CH], BF, tag="pt", name=f"pt{h}_{c}_{kt}")
                    nc.scalar.activation(pt[:, qo:], ps[:, qo:],
                                         mybir.ActivationFunctionType.Exp,
                                         scale=SCALE)
                    nc.tensor.matmul(pd[:, qo:], ones_col[:], pt[:, qo:],
                                     start=(kt == 0), stop=(kt == nkt - 1))
                    nc.tensor.matmul(po[:, qo:],
                                     VV[:, kt * 256 + h * 128: kt * 256 + (h + 1) * 128],
                                     pt[:, qo:],
                                     start=(kt == 0), stop=(kt == nkt - 1))
                # normalize: AT = po * (1/pd), broadcast via GPSIMD (off PE)
                rcp = smallp.tile([1, CH], F32, tag="rcp", name=f"rcp{h}_{c}")
                nc.vector.reciprocal(rcp[:], pd[:])
                binv = smallp.tile([128, CH], F32, tag="binv", name=f"binv{h}_{c}")
                nc.gpsimd.partition_broadcast(binv[:], rcp[:])
                nc.vector.tensor_mul(AT[h][c][:], po[:], binv[:])

            # ---- output projection, one chunk behind attention ----
            for j in (range(4) if c >= 1 else []):
                _emit_wo_tile(nc, pswo, stgp, AT, wo_sb, out, c - 1, j)
        for j in range(4):
            _emit_wo_tile(nc, pswo, stgp, AT, wo_sb, out, NCH - 1, j)


_NC_CACHE = None


def _get_nc():
    global _NC_CACHE
    if _NC_CACHE is None:
        _NC_CACHE = _build_bass()
    return _NC_CACHE


def _f16(x):
    return np.ascontiguousarray(x).astype(np.float16)


def prepare_inputs(hidden_states, positions, w_qkv, w_o):
    """Host-side sharding/preprocessing -> list of per-core input maps."""
    hidden_states = np.asarray(hidden_states, dtype=np.float32)
    positions = np.asarray(positions)
    w_qkv = np.asarray(w_qkv, dtype=np.float32)
    w_o = np.asarray(w_o, dtype=np.float32)

    hidT_f16 = _f16(hidden_states.T)

    pos = positions.astype(np.float32)
    half = HD // 2
    inv_freq = 1.0 / (10000.0 ** (np.arange(half, dtype=np.float32) / half))
    freqs = np.outer(pos, inv_freq)          # [T, 64]
    cos = np.cos(freqs).T                    # [64, T]
    sin = np.sin(freqs).T
    cos2 = _f16(np.concatenate([cos, cos], axis=0))
    sin2 = _f16(np.concatenate([-sin, sin], axis=0))

    # causal masks for the 4 diagonal alignments: block r: 0 where 128r+k <= q
    k_idx = np.arange(128)[:, None]
    q_idx = np.arange(CH)[None, :]
    mblocks = [np.where(128 * r + k_idx <= q_idx, 0.0, MASK_NEG).astype(np.float32)
               for r in range(4)]
    masks_np = np.concatenate(mblocks, axis=1)

    in_maps = []
    for core in range(NCORES):
        heads = [HPC * core + i for i in range(HPC)]
        wq = [w_qkv[:, h * HD:(h + 1) * HD] for h in heads]
        wk = [w_qkv[:, FEAT * NCORES + h * HD:FEAT * NCORES + (h + 1) * HD]
              for h in heads]
        wv = [w_qkv[:, 2 * FEAT * NCORES + h * HD:2 * FEAT * NCORES + (h + 1) * HD]
              for h in heads]
        wqkv_core = _f16(np.concatenate(wq + wk + wv, axis=1))
        wo_core = _f16(np.concatenate(
            [w_o[h * HD:(h + 1) * HD, :] for h in heads], axis=0))
        in_maps.append({
            "hidT": hidT_f16,
            "wqkv": wqkv_core,
            "wo": wo_core,
            "cos2": cos2,
            "sin2": sin2,
            "masks": masks_np,
        })
    return in_maps


def kernel(hidden_states, positions, w_qkv, w_o):
    in_maps = prepare_inputs(hidden_states, positions, w_qkv, w_o)
    nc = _get_nc()
    try:
        res = run_bass_kernel_spmd(nc, in_maps, core_ids=list(range(NCORES)))
    except Exception:
        # transient device wedge from a prior crashed process: retry once
        res = run_bass_kernel_spmd(nc, in_maps, core_ids=list(range(NCORES)))
    acc = res.results[0]["out"].astype(np.float32)
    for i in range(1, NCORES):
        acc += res.results[i]["out"].astype(np.float32)
    return acc

def _emit_wo_tile(nc, pswo, stgp, AT, wo_sb, out, c, j):
    """w_o projection for T-tile tt = 4c+j: 4 n-chunks of 512 columns."""
    tt = 4 * c + j
    stg = stgp.tile([128, HID], F16, tag="stg", bufs=3, name=f"stg{tt}")
    for n in range(HID // CH):
        pw = pswo.tile([128, CH], F32, tag="mm", name=f"pw{tt}_{n}")
        for h in range(HPC):
            nc.tensor.matmul(
                pw[:],
                AT[h][c][:, j * 128:(j + 1) * 128],
                wo_sb[h][:, n * CH:(n + 1) * CH],
                start=(h == 0), stop=(h == HPC - 1))
        if n % 2 == 0:
            nc.vector.tensor_copy(stg[:, n * CH:(n + 1) * CH], pw[:])
        else:
            nc.scalar.copy(stg[:, n * CH:(n + 1) * CH], pw[:])
    eng = nc.sync if tt % 2 == 0 else nc.scalar
    eng.dma_start(out[tt * 128:(tt + 1) * 128, :], stg[:])
